# revision 12
# baseline (speedup 1.0000x reference)
"""Trainium2 Bass kernel for the CWRRT cell (rolling-memory cross-attention + MLP + EMA).

Self-contained: builds an SPMD Bass/Tile program, shards the batch over 8
NeuronCores, runs via run_bass_kernel_spmd, and gathers full-shape outputs.

Math (per batch row b):
  x_in  = x + delta
  mem'  = [mem[1:16], x_in]                       (rolling KV cache)
  q     = LN1(x_in) @ Wq_eff + bq_eff             (LN scale/bias + 1/sqrt(D) folded in)
  K_m   = mem'_m @ Wk        (bk dropped: softmax-invariant)
  logits[h,m] = q_h . K_m_h ; attn = softmax_m
  o     = sum_m attn[:,m] * (mem'_m @ Wv)         (bv folded into bo_eff)
  x_mid = x_in + o @ Wo + bo_eff
  h     = gelu_tanh(LN2(x_mid) @ W1_eff + b1_eff)
  x_out = x_mid + h @ W2 + b2
  nssum = 0.9 ssum + 0.1 x_out

Layout: batch rows blocked "(p t)": SBUF partition p owns rows 8p..8p+7 of the
core's shard; b-tile t = one row per partition. All activations are b-major
[128, feat]; GEMMs use PE-transposed bf16 activation chunks as the stationary
operand and natural-layout bf16 weights as the moving operand, so outputs land
b-major in PSUM (no partition shuffles anywhere).
"""
import sys
import os

sys.path.insert(0, '/opt/trn_rl_repo')

import numpy as np
import ml_dtypes
from contextlib import ExitStack

import concourse.bass as bass
import concourse.mybir as mybir
import concourse.tile as tile
from concourse import bacc
from concourse.masks import make_identity

fp32 = mybir.dt.float32
bf16 = mybir.dt.bfloat16
AF = mybir.ActivationFunctionType
ALU = mybir.AluOpType
X = mybir.AxisListType.X

B, E, H, M, D = 8192, 512, 8, 16, 64
NCORES = 8
BLOC = B // NCORES          # rows per core
P = 128
EPS = 1e-6
LAM = 0.9
EC = E // P                 # 4 e-chunks
FC = (4 * E) // P           # 16 f-chunks (MLP hidden)


def _bc(t: bass.AP, dims):
    """Read-AP over tile t with explicit free [step, count] dims (step 0 = broadcast)."""
    return bass.AP(tensor=t.tensor, offset=t.offset, ap=[t.ap[0]] + dims)


def build_program(bloc=BLOC):
    """Build the SPMD Bass program for one core processing `bloc` batch rows."""
    T = bloc // P               # b-tiles (slots per partition)
    nc = bacc.Bacc("TRN2", target_bir_lowering=False, debug=False)

    # ---- DRAM I/O ----
    d_mem = nc.dram_tensor("mem", [bloc, M, E], fp32, kind="ExternalInput").ap()
    d_x = nc.dram_tensor("x", [bloc, E], fp32, kind="ExternalInput").ap()
    d_delta = nc.dram_tensor("delta", [bloc, E], fp32, kind="ExternalInput").ap()
    d_ssum = nc.dram_tensor("ssum", [bloc, E], fp32, kind="ExternalInput").ap()
    d_wq = nc.dram_tensor("wq", [E, E], bf16, kind="ExternalInput").ap()
    d_wk = nc.dram_tensor("wk", [E, E], bf16, kind="ExternalInput").ap()
    d_wv = nc.dram_tensor("wv", [E, E], bf16, kind="ExternalInput").ap()
    d_wo = nc.dram_tensor("wo", [E, E], bf16, kind="ExternalInput").ap()
    d_w1 = nc.dram_tensor("w1", [E, 4 * E], bf16, kind="ExternalInput").ap()
    d_w2 = nc.dram_tensor("w2", [4 * E, E], bf16, kind="ExternalInput").ap()
    d_bq = nc.dram_tensor("bq", [E], fp32, kind="ExternalInput").ap()
    d_bo = nc.dram_tensor("bo", [E], fp32, kind="ExternalInput").ap()
    d_b1 = nc.dram_tensor("b1", [4 * E], fp32, kind="ExternalInput").ap()
    d_b2 = nc.dram_tensor("b2", [E], fp32, kind="ExternalInput").ap()
    d_upd = nc.dram_tensor("upd", [bloc, M, E], fp32, kind="ExternalOutput").ap()
    d_nss = nc.dram_tensor("nssum", [bloc, E], fp32, kind="ExternalOutput").ap()
    d_xout = nc.dram_tensor("xout", [bloc, E], fp32, kind="ExternalOutput").ap()

    # blocked views: partition p <-> rows T*p .. T*p+T-1
    v_mem = d_mem.rearrange("(p t) m e -> p t m e", p=P)
    v_x = d_x.rearrange("(p t) e -> p t e", p=P)
    v_delta = d_delta.rearrange("(p t) e -> p t e", p=P)
    v_ssum = d_ssum.rearrange("(p t) e -> p t e", p=P)
    v_upd = d_upd.rearrange("(p t) m e -> p t m e", p=P)
    v_nss = d_nss.rearrange("(p t) e -> p t e", p=P)
    v_xout = d_xout.rearrange("(p t) e -> p t e", p=P)
    # weight views: partition p row = e-chunk c, e = c*128 + p
    v_wq = d_wq.rearrange("(c p) n -> p c n", p=P)
    v_wk = d_wk.rearrange("(c p) n -> p c n", p=P)
    v_wv = d_wv.rearrange("(c p) n -> p c n", p=P)
    v_wo = d_wo.rearrange("(c p) n -> p c n", p=P)
    v_w1 = d_w1.rearrange("(c p) n -> p c n", p=P)
    v_w2 = d_w2.rearrange("(c p) n -> p c n", p=P)

    with tile.TileContext(nc) as tc, ExitStack() as ctx:
        consts = ctx.enter_context(tc.tile_pool(name="consts", bufs=1))
        memq = ctx.enter_context(tc.tile_pool(name="memq", bufs=3))
        memTp = ctx.enter_context(tc.tile_pool(name="memT", bufs=2))
        work = ctx.enter_context(tc.tile_pool(name="work", bufs=2))
        wsmall = ctx.enter_context(tc.tile_pool(name="wsmall", bufs=2))
        ps_tp = ctx.enter_context(tc.tile_pool(name="ps_tp", bufs=2, space="PSUM"))
        ps_kv = ctx.enter_context(tc.tile_pool(name="ps_kv", bufs=2, space="PSUM"))
        ps_mm = ctx.enter_context(tc.tile_pool(name="ps_mm", bufs=2, space="PSUM"))
        ps_h = ctx.enter_context(tc.tile_pool(name="ps_h", bufs=2, space="PSUM"))

        # ---- constants ----
        wq_sb = consts.tile([P, EC, E], bf16)
        nc.sync.dma_start(wq_sb[:], v_wq)
        wk_sb = consts.tile([P, EC, E], bf16)
        nc.sync.dma_start(wk_sb[:], v_wk)
        wv_sb = consts.tile([P, EC, E], bf16)
        nc.sync.dma_start(wv_sb[:], v_wv)
        wo_sb = consts.tile([P, EC, E], bf16)
        nc.sync.dma_start(wo_sb[:], v_wo)
        w1_sb = consts.tile([P, EC, 4 * E], bf16)
        nc.sync.dma_start(w1_sb[:], v_w1)
        w2_sb = consts.tile([P, FC, E], bf16)
        nc.sync.dma_start(w2_sb[:], v_w2)

        def bias_tile(dram_ap, n, tag, dt=fp32):
            t = consts.tile([P, n], dt, tag=tag)
            bcast = bass.AP(tensor=dram_ap.tensor, offset=dram_ap.offset,
                            ap=[[0, P], dram_ap.ap[0]])
            nc.sync.dma_start(t[:], bcast)
            return t

        bq_sb = bias_tile(d_bq, E, "bq")
        bo_sb = bias_tile(d_bo, E, "bo")
        b1_sb = bias_tile(d_b1, 4 * E, "b1")
        b2_sb = bias_tile(d_b2, E, "b2")
        ident = consts.tile([P, P], bf16)
        make_identity(nc, ident[:])
        eps_sb = consts.tile([P, 1], fp32)
        nc.vector.memset(eps_sb[:], EPS)

        def transpose_to(dst_sbuf, src_bf16, nblk):
            """PE-transpose nblk [128,128] bf16 blocks of src into dst sbuf [128, nblk, 128]."""
            done = 0
            while done < nblk:
                g = min(4, nblk - done)
                tp = ps_tp.tile([P, 4, P], bf16, tag="tp")
                for j in range(g):
                    nc.tensor.transpose(
                        tp[:, j, :], src_bf16[:, (done + j) * P:(done + j + 1) * P],
                        ident[:])
                nc.scalar.copy(dst_sbuf[:, done:done + g, :], tp[:, 0:g, :])
                done += g

        def layernorm_bf16(dst_bf16, src_f32):
            """dst = (src - mean)/sqrt(var+eps), cast to bf16."""
            st = wsmall.tile([P, 6], fp32, tag="lnst")
            nc.vector.bn_stats(st[:], src_f32[:])
            mv = wsmall.tile([P, 2], fp32, tag="lnmv")
            nc.vector.bn_aggr(mv[:], st[:])
            rs = wsmall.tile([P, 1], fp32, tag="lnrs")
            nc.scalar.activation(rs[:], mv[:, 1:2], AF.Sqrt, bias=eps_sb[:], scale=1.0)
            nc.vector.reciprocal(rs[:], rs[:])
            nc.vector.tensor_scalar(dst_bf16[:], src_f32[:], mv[:, 0:1], rs[:],
                                    ALU.subtract, ALU.mult)

        for t in range(T):
            # ---- loads ----
            x_t = work.tile([P, E], fp32, tag="x")
            nc.sync.dma_start(x_t[:], v_x[:, t, :])
            dl_t = work.tile([P, E], fp32, tag="dl")
            nc.sync.dma_start(dl_t[:], v_delta[:, t, :])
            ss_t = work.tile([P, E], fp32, tag="ss")
            nc.sync.dma_start(ss_t[:], v_ssum[:, t, :])

            x_in = work.tile([P, E], fp32, tag="xin")
            nc.vector.tensor_add(x_in[:], x_t[:], dl_t[:])
            nc.sync.dma_start(v_upd[:, t, M - 1, :], x_in[:])

            # ---- mem': load quarters, shift-store, cast, transpose ----
            memT = memTp.tile([P, M, EC, P], bf16)  # [e, (m, ec, b)]
            for qd in range(4):
                lo = 4 * qd            # mem' slot range [lo, hi)
                hi = min(4 * qd + 4, M - 1)
                mq = memq.tile([P, 4, E], fp32, tag="mq")
                nc.sync.dma_start(mq[:, 0:hi - lo, :], v_mem[:, t, lo + 1:hi + 1, :])
                nc.sync.dma_start(v_upd[:, t, lo:hi, :], mq[:, 0:hi - lo, :])
                for j in range(hi - lo):
                    mb = wsmall.tile([P, E], bf16, tag="mb")
                    nc.gpsimd.tensor_copy(mb[:], mq[:, j, :])
                    transpose_to(memT[:, lo + j], mb[:], EC)
            xin_b = wsmall.tile([P, E], bf16, tag="mb")
            nc.gpsimd.tensor_copy(xin_b[:], x_in[:])
            transpose_to(memT[:, M - 1], xin_b[:], EC)

            # ---- LN1 + Q ----
            xh = wsmall.tile([P, E], bf16, tag="xh")
            layernorm_bf16(xh, x_in)
            xhT = wsmall.tile([P, EC, P], bf16, tag="xhT")
            transpose_to(xhT, xh[:], EC)
            q_ps = ps_mm.tile([P, E], fp32, tag="mm")
            for ec in range(EC):
                nc.tensor.matmul(q_ps[:], lhsT=xhT[:, ec, :], rhs=wq_sb[:, ec, :],
                                 start=(ec == 0), stop=(ec == EC - 1))
            q_sb = work.tile([P, E], fp32, tag="q")
            nc.vector.tensor_add(q_sb[:], q_ps[:], bq_sb[:])

            # ---- K-pass: logits[m, h] ----
            logits = work.tile([P, M, H], fp32, tag="lg")
            for m in range(M):
                kps = ps_kv.tile([P, E], fp32, tag="kv")
                for ec in range(EC):
                    nc.tensor.matmul(kps[:], lhsT=memT[:, m, ec, :],
                                     rhs=wk_sb[:, ec, :],
                                     start=(ec == 0), stop=(ec == EC - 1))
                tmp = wsmall.tile([P, E], fp32, tag="tmp")
                nc.vector.tensor_mul(tmp[:], kps[:], q_sb[:])
                nc.vector.reduce_sum(logits[:, m, :],
                                     tmp[:].rearrange("p (h d) -> p h d", h=H),
                                     axis=X)

            # ---- softmax over m ----
            mx = wsmall.tile([P, H], fp32, tag="mx")
            nc.vector.reduce_max(mx[:], logits[:].rearrange("p m h -> p h m"), axis=X)
            ex = work.tile([P, M, H], fp32, tag="ex")
            nc.vector.tensor_sub(ex[:], logits[:], _bc(mx, [[0, M], [1, H]]))
            nc.scalar.activation(ex[:], ex[:], AF.Exp)
            sm = wsmall.tile([P, H], fp32, tag="sm")
            nc.vector.reduce_sum(sm[:], ex[:].rearrange("p m h -> p h m"), axis=X)
            rc = wsmall.tile([P, H], fp32, tag="rc")
            nc.vector.reciprocal(rc[:], sm[:])
            attn = work.tile([P, M, H], fp32, tag="at")
            nc.vector.tensor_mul(attn[:], ex[:], _bc(rc, [[0, M], [1, H]]))

            # ---- V-pass: o = sum_m attn_m * V_m ----
            o_sb = work.tile([P, E], fp32, tag="o")
            for m in range(M):
                vps = ps_kv.tile([P, E], fp32, tag="kv")
                for ec in range(EC):
                    nc.tensor.matmul(vps[:], lhsT=memT[:, m, ec, :],
                                     rhs=wv_sb[:, ec, :],
                                     start=(ec == 0), stop=(ec == EC - 1))
                # attn[:, m, :] broadcast over d: offset m*H, dims [h(step1), d(step0)]
                am = bass.AP(tensor=attn.tensor, offset=attn.offset + m * H,
                             ap=[attn.ap[0], [1, H], [0, D]])
                vv = vps[:].rearrange("p (h d) -> p h d", h=H)
                if m == 0:
                    nc.vector.tensor_tensor(
                        o_sb[:].rearrange("p (h d) -> p h d", h=H), vv, am, ALU.mult)
                else:
                    pm = wsmall.tile([P, E], fp32, tag="pm")
                    nc.vector.tensor_tensor(
                        pm[:].rearrange("p (h d) -> p h d", h=H), vv, am, ALU.mult)
                    nc.vector.tensor_add(o_sb[:], o_sb[:], pm[:])

            # ---- O-proj + residual ----
            ob = wsmall.tile([P, E], bf16, tag="ob")
            nc.gpsimd.tensor_copy(ob[:], o_sb[:])
            oT = wsmall.tile([P, EC, P], bf16, tag="oT")
            transpose_to(oT, ob[:], EC)
            ya_ps = ps_mm.tile([P, E], fp32, tag="mm")
            for ec in range(EC):
                nc.tensor.matmul(ya_ps[:], lhsT=oT[:, ec, :], rhs=wo_sb[:, ec, :],
                                 start=(ec == 0), stop=(ec == EC - 1))
            x_mid = work.tile([P, E], fp32, tag="xm")
            nc.vector.tensor_add(x_mid[:], ya_ps[:], bo_sb[:])
            nc.vector.tensor_add(x_mid[:], x_mid[:], x_in[:])

            # ---- LN2 + MLP ----
            y2h = wsmall.tile([P, E], bf16, tag="xh")
            layernorm_bf16(y2h, x_mid)
            y2T = wsmall.tile([P, EC, P], bf16, tag="xhT")
            transpose_to(y2T, y2h[:], EC)
            hb = work.tile([P, FC, P], bf16, tag="hb")   # gelu output [128, 2048] bf16
            for fc in range(4):
                hps = ps_h.tile([P, E], fp32, tag="h")
                for ec in range(EC):
                    nc.tensor.matmul(hps[:], lhsT=y2T[:, ec, :],
                                     rhs=w1_sb[:, ec, fc * E:(fc + 1) * E],
                                     start=(ec == 0), stop=(ec == EC - 1))
                nc.vector.tensor_add(hps[:], hps[:], b1_sb[:, fc * E:(fc + 1) * E])
                nc.scalar.activation(
                    hb[:].rearrange("p f b -> p (f b)")[:, fc * E:(fc + 1) * E],
                    hps[:], AF.Gelu_apprx_tanh)
            hT = wsmall.tile([P, FC, P], bf16, tag="hT")
            transpose_to(hT, hb[:].rearrange("p f b -> p (f b)"), FC)
            y3_ps = ps_mm.tile([P, E], fp32, tag="mm")
            for k in range(FC):
                nc.tensor.matmul(y3_ps[:], lhsT=hT[:, k, :], rhs=w2_sb[:, k, :],
                                 start=(k == 0), stop=(k == FC - 1))

            # ---- x_out, EMA, stores ----
            xo = work.tile([P, E], fp32, tag="xo")
            nc.vector.tensor_add(xo[:], y3_ps[:], b2_sb[:])
            nc.vector.tensor_add(xo[:], xo[:], x_mid[:])
            nc.sync.dma_start(v_xout[:, t, :], xo[:])
            nss = work.tile([P, E], fp32, tag="ns")
            nc.vector.tensor_sub(nss[:], ss_t[:], xo[:])
            nc.vector.scalar_tensor_tensor(nss[:], nss[:], LAM, xo[:],
                                           ALU.mult, ALU.add)
            nc.sync.dma_start(v_nss[:, t, :], nss[:])

    nc.compile()
    return nc


_PROGRAM_CACHE = {}


def _get_program(bloc):
    if bloc not in _PROGRAM_CACHE:
        _PROGRAM_CACHE[bloc] = build_program(bloc)
    return _PROGRAM_CACHE[bloc]


def preprocess_weights(inputs):
    """Host-side weight folding (numpy). Returns dict of per-core-replicated arrays."""
    f32 = np.float32
    wq = inputs["wq"].reshape(E, E).astype(f32)
    wk = inputs["wk"].reshape(E, E).astype(f32)
    wv = inputs["wv"].reshape(E, E).astype(f32)
    wo = inputs["wo"].reshape(E, E).astype(f32)
    bq = inputs["bq"].reshape(E).astype(f32)
    bv = inputs["bv"].reshape(E).astype(f32)
    bo = inputs["bo"].reshape(E).astype(f32)
    w1 = inputs["w1"].astype(f32)
    b1 = inputs["b1"].astype(f32)
    w2 = inputs["w2"].astype(f32)
    b2 = inputs["b2"].astype(f32)
    s1 = inputs["ln1_scale"].astype(f32)
    c1 = inputs["ln1_bias"].astype(f32)
    s2 = inputs["ln2_scale"].astype(f32)
    c2 = inputs["ln2_bias"].astype(f32)
    isq = 1.0 / np.sqrt(np.float32(D))

    wq_eff = (s1[:, None] * wq) * isq
    bq_eff = (c1 @ wq + bq) * isq
    bo_eff = bv @ wo + bo
    w1_eff = s2[:, None] * w1
    b1_eff = c2 @ w1 + b1

    bf = ml_dtypes.bfloat16
    return {
        "wq": wq_eff.astype(bf), "wk": wk.astype(bf), "wv": wv.astype(bf),
        "wo": wo.astype(bf), "w1": w1_eff.astype(bf), "w2": w2.astype(bf),
        "bq": bq_eff.astype(f32), "bo": bo_eff.astype(f32),
        "b1": b1_eff.astype(f32), "b2": b2.astype(f32),
    }


def prepare_in_maps(inputs):
    wmap = preprocess_weights(inputs)
    mem = np.ascontiguousarray(inputs["mem"], dtype=np.float32)
    x = np.ascontiguousarray(inputs["x"], dtype=np.float32)
    delta = np.ascontiguousarray(inputs["delta"], dtype=np.float32)
    ssum = np.ascontiguousarray(inputs["ssum"], dtype=np.float32)

    in_maps = []
    for c in range(NCORES):
        sl = slice(c * BLOC, (c + 1) * BLOC)
        in_maps.append({
            "mem": mem[sl], "x": x[sl], "delta": delta[sl], "ssum": ssum[sl],
            **wmap,
        })
    return in_maps


def kernel(**inputs):
    from concourse.bass_utils import run_bass_kernel_spmd

    nc = _get_program(BLOC)
    in_maps = prepare_in_maps(inputs)
    res = run_bass_kernel_spmd(nc, in_maps, list(range(NCORES)))
    upd = np.concatenate([res.results[c]["upd"] for c in range(NCORES)], axis=0)
    nss = np.concatenate([res.results[c]["nssum"] for c in range(NCORES)], axis=0)
    xout = np.concatenate([res.results[c]["xout"] for c in range(NCORES)], axis=0)
    return upd, nss, xout


if __name__ == "__main__":
    rng = np.random.default_rng(0)
    ins = {
        "mem": rng.standard_normal((B, M, E), dtype=np.float32),
        "ssum": rng.standard_normal((B, E), dtype=np.float32),
        "x": rng.standard_normal((B, E), dtype=np.float32),
        "delta": rng.standard_normal((B, E), dtype=np.float32),
        "ln1_scale": np.ones(E, np.float32), "ln1_bias": np.zeros(E, np.float32),
        "wq": rng.standard_normal((E, H, D), dtype=np.float32) * 0.02,
        "bq": np.zeros((H, D), np.float32),
        "wk": rng.standard_normal((E, H, D), dtype=np.float32) * 0.02,
        "bk": np.zeros((H, D), np.float32),
        "wv": rng.standard_normal((E, H, D), dtype=np.float32) * 0.02,
        "bv": np.zeros((H, D), np.float32),
        "wo": rng.standard_normal((H, D, E), dtype=np.float32) * 0.02,
        "bo": np.zeros(E, np.float32),
        "ln2_scale": np.ones(E, np.float32), "ln2_bias": np.zeros(E, np.float32),
        "w1": rng.standard_normal((E, 4 * E), dtype=np.float32) * 0.02,
        "b1": np.zeros(4 * E, np.float32),
        "w2": rng.standard_normal((4 * E, E), dtype=np.float32) * 0.02,
        "b2": np.zeros(E, np.float32),
    }
    outs = kernel(**ins)
    print([o.shape for o in outs])
